# revision 20
# baseline (speedup 1.0000x reference)
"""Trainium2 Bass kernel for nn_ExemplarSoftmaxLoss (data-parallel over 8 cores).

Strategy:
  - Shard batch dim B (and the 3 B-row blocks of `outputs`) across 8 cores.
  - Per core, on device:
      * softmax part: per-row sum(exp(x)) via ScalarE Exp with row-accumulate
        (no max subtraction needed: |x| <= ~6 so exp is safely in fp32 range);
        label logits extracted on VectorE with a fused
        (iota == label) * x row-sum (scalar_tensor_tensor with accum_out).
      * distance part: exemplar rows gathered per 128-row block with
        [128,1]-offset indirect DMAs (the only HW-verified gather pattern),
        diffs (x + eps - y) on VectorE, squared-row-sums on ScalarE
        (Square w/ accumulate), sqrt + hinge logic on tiny [128,16] tiles.
      * outputs per-partition partial sums [128, 4].
  - Host: float64 reduction of the 8x[128,4] partials -> 4 scalar losses.
"""

import os
import sys

import numpy as np

for _p in ("/opt/trn_rl_repo",):
    if _p not in sys.path and os.path.isdir(_p):
        sys.path.insert(0, _p)

import concourse.bass as bass
import concourse.tile as tile
from concourse import bacc, mybir
from concourse._compat import with_exitstack
from concourse.bass_utils import run_bass_kernel_spmd

# Problem constants (hardcoded per the harness contract).
B, D, C = 16384, 512, 1000
NCORES = 8
BS = B // NCORES  # 2048 batch rows per core
RS = 3 * BS  # 6144 softmax rows per core
P = 128
NB = BS // P  # 16 row-blocks in the distance phase
NR = RS // P  # 48 row-blocks in the softmax phase
NG = 4  # groups of 4 row-blocks in the distance phase
EPS = 1e-6
MARGIN2 = 0.2
LAMBDA = 1.0

f32 = mybir.dt.float32
i32 = mybir.dt.int32
Alu = mybir.AluOpType
Act = mybir.ActivationFunctionType
AX = mybir.AxisListType

LAST_RESULTS = None  # BassKernelResults of the most recent run (for test.py)


@with_exitstack
def _emit(ctx, tc, outs, ins):
    nc = tc.nc
    xo = ins["xout"]  # [RS, C]   f32 outputs shard (3 blocks concatenated)
    aa = ins["anc"]  # [BS, D]   f32
    pp = ins["pos"]  # [BS, D]   f32
    ng = ins["neg"]  # [BS, D]   f32
    ex = ins["exem"]  # [C, D]    f32 exemplar table
    la = ins["lab_a"]  # [P, NB]  i32  labels_anchor, row blk*128+p at [p, blk]
    ln = ins["lab_n"]  # [P, NB]  i32  labels_neg
    lf = ins["lab_f"]  # [P, NR]  f32  concat labels as f32, row rb*128+p at [p, rb]
    pd = outs["partials"]  # [P, 4]  f32

    sing = ctx.enter_context(tc.tile_pool(name="sing", bufs=1))
    xpool = ctx.enter_context(tc.tile_pool(name="xp", bufs=5))
    ejp = ctx.enter_context(tc.tile_pool(name="ejp", bufs=2))
    ljp = ctx.enter_context(tc.tile_pool(name="ljp", bufs=2))
    apnp = ctx.enter_context(tc.tile_pool(name="apnp", bufs=2))
    expool = ctx.enter_context(tc.tile_pool(name="expool", bufs=2))
    dfp = ctx.enter_context(tc.tile_pool(name="dfp", bufs=3))
    sqp = ctx.enter_context(tc.tile_pool(name="sqp", bufs=3))

    sums = sing.tile([P, NR], f32)  # per-row sum(exp(x))
    lbl = sing.tile([P, NR], f32)  # extracted label logits
    d2a = sing.tile([P, NB * 3], f32)  # sq dists: dr1,dn1,dr2
    d2v = sing.tile([P, NB * 3], f32)  # sq dists: dn2,tp,tn
    la_t = sing.tile([P, NB], i32)
    ln_t = sing.tile([P, NB], i32)
    lf_t = sing.tile([P, NR], f32)
    iota_t = sing.tile([P, C], f32)

    # small loads via SWDGE so the Sync HWDGE queue leads with the x-tile stream
    nc.gpsimd.dma_start(out=la_t[:], in_=la[:])
    nc.gpsimd.dma_start(out=ln_t[:], in_=ln[:])
    nc.gpsimd.dma_start(out=lf_t[:], in_=lf[:])
    nc.gpsimd.iota(
        iota_t[:],
        pattern=[[1, C]],
        base=0,
        channel_multiplier=0,
        allow_small_or_imprecise_dtypes=True,
    )

    def emit_gathers(g, exa, exn, b2s=range(4)):
        for b2 in b2s:
            blk = 4 * g + b2
            nc.gpsimd.indirect_dma_start(
                out=exa[:, b2, :],
                out_offset=None,
                in_=ex[:],
                in_offset=bass.IndirectOffsetOnAxis(ap=la_t[:, blk : blk + 1], axis=0),
            )
            nc.gpsimd.indirect_dma_start(
                out=exn[:, b2, :],
                out_offset=None,
                in_=ex[:],
                in_offset=bass.IndirectOffsetOnAxis(ap=ln_t[:, blk : blk + 1], axis=0),
            )

    def emit_apn_loads(g):
        at = apnp.tile([P, 4, D], f32, tag="at", name=f"at{g}")
        pt = apnp.tile([P, 4, D], f32, tag="pt", name=f"pt{g}")
        nt = apnp.tile([P, 4, D], f32, tag="nt", name=f"nt{g}")
        r0, r1 = g * 4 * P, (g + 1) * 4 * P
        nc.sync.dma_start(
            out=at[:], in_=aa[r0:r1, :].rearrange("(t p) d -> p t d", p=P)
        )
        nc.sync.dma_start(
            out=pt[:], in_=pp[r0:r1, :].rearrange("(t p) d -> p t d", p=P)
        )
        nc.sync.dma_start(
            out=nt[:], in_=ng[r0:r1, :].rearrange("(t p) d -> p t d", p=P)
        )
        return at, pt, nt

    # software-pipeline the exemplar gathers + anchor/pos/neg loads one group
    # ahead so SWDGE descriptor generation and DMA overlap compute
    ex_tiles = {
        0: (
            expool.tile([P, 4, D], f32, tag="exa", name="exa0"),
            expool.tile([P, 4, D], f32, tag="exn", name="exn0"),
        )
    }
    emit_gathers(0, *ex_tiles[0])
    apn_tiles = {}

    for g in range(NG):
        exa, exn = ex_tiles.pop(g)
        # interleave 1 x-tile : 1 distance pair for smooth per-engine FIFOs
        for pi in range(6):
            xi = 6 * g + pi
            xt = xpool.tile([P, 2, C], f32, tag="xt")
            nc.sync.dma_start(
                out=xt[:],
                in_=xo[xi * 2 * P : (xi + 1) * 2 * P, :].rearrange(
                    "(t p) c -> p t c", p=P
                ),
            )
            for b in range(2):
                rb = xi * 2 + b
                ej = ejp.tile([P, C], f32, tag="ej")
                nc.scalar.activation(
                    out=ej[:],
                    in_=xt[:, b, :],
                    func=Act.Exp,
                    accum_out=sums[:, rb : rb + 1],
                )
                # lbl[p, rb] = sum((iota == label) * x) = x[p, label]
                lj = ljp.tile([P, C], f32, tag="lj")
                nc.vector.scalar_tensor_tensor(
                    out=lj[:],
                    in0=iota_t[:],
                    scalar=lf_t[:, rb : rb + 1],
                    in1=xt[:, b, :],
                    op0=Alu.is_equal,
                    op1=Alu.mult,
                    accum_out=lbl[:, rb : rb + 1],
                )

            if g == 0 and pi == 0:
                # first group's apn loads queue behind the first x-tile
                apn_tiles[0] = emit_apn_loads(0)
            if pi == 2 and g + 1 < NG:
                ex_tiles[g + 1] = (
                    expool.tile([P, 4, D], f32, tag="exa", name=f"exa{g + 1}"),
                    expool.tile([P, 4, D], f32, tag="exn", name=f"exn{g + 1}"),
                )
            if pi == 3 and g + 1 < NG:
                # prefetch next group's apn mid-group (decongests the ramp)
                apn_tiles[g + 1] = emit_apn_loads(g + 1)
            if pi >= 2 and g + 1 < NG:
                # spread next group's gathers: 2 indirect DMAs per step
                emit_gathers(g + 1, *ex_tiles[g + 1], b2s=[pi - 2])

            if pi == 0:
                at, pt, nt = apn_tiles.pop(g)
                # squares: ~68 on ScalarE, ~28 on VectorE (measured balance);
                # dn2/tp partially alternate per group
                pairs = (
                    (at[:], exa[:], d2a, 0, True),  # d_ref1  -> ScalarE
                    (nt[:], exa[:], d2a, 1, True),  # d_neg1  -> ScalarE
                    (at[:], exn[:], d2a, 2, True),  # d_ref2  -> ScalarE
                    (nt[:], exn[:], d2v, 0, g != 3),  # d_neg2 -> 12/4
                    (at[:], pt[:], d2v, 1, g % 2 == 0),  # tp -> split 8/8
                    (at[:], nt[:], d2v, 2, False),  # tn    -> VectorE
                )
            xs, ys, d2t, ci, on_act = pairs[pi]

            df = dfp.tile([P, 4, D], f32, tag="df")
            # df = x - y.  (The reference's +EPS inside the norm shifts d^2
            # by ~2*EPS*|sum(diff)| ~ 1e-7 relative -- negligible.)
            nc.vector.tensor_tensor(out=df[:], in0=xs, in1=ys, op=Alu.subtract)
            for b2 in range(4):
                col = (4 * g + b2) * 3 + ci
                if on_act:
                    sq = sqp.tile([P, D], f32, tag="sqa")
                    nc.scalar.activation(
                        out=sq[:],
                        in_=df[:, b2, :],
                        func=Act.Square,
                        accum_out=d2t[:, col : col + 1],
                    )
                else:
                    sq = sqp.tile([P, D], f32, tag="sqv")
                    # (df * 1.0) * df with sum-accumulate == row-sum of df^2
                    nc.vector.scalar_tensor_tensor(
                        out=sq[:],
                        in0=df[:, b2, :],
                        scalar=1.0,
                        in1=df[:, b2, :],
                        op0=Alu.mult,
                        op1=Alu.mult,
                        accum_out=d2t[:, col : col + 1],
                    )

    # ---- tail ----
    part = sing.tile([P, 4], f32)
    logs = sing.tile([P, NR], f32)
    nc.scalar.activation(out=logs[:], in_=sums[:], func=Act.Ln)
    nc.vector.reduce_sum(out=part[:, 0:1], in_=logs[:], axis=AX.X)
    nc.vector.reduce_sum(out=part[:, 1:2], in_=lbl[:], axis=AX.X)

    dda = sing.tile([P, NB * 3], f32)
    ddv = sing.tile([P, NB * 3], f32)
    nc.scalar.activation(out=dda[:], in_=d2a[:], func=Act.Sqrt)
    nc.scalar.activation(out=ddv[:], in_=d2v[:], func=Act.Sqrt)
    dA = dda[:].rearrange("p (b k) -> p b k", k=3)
    dV = ddv[:].rearrange("p (b k) -> p b k", k=3)

    x1 = sing.tile([P, NB], f32)
    m1 = sing.tile([P, NB], f32)
    c1 = sing.tile([P, NB], f32)
    x2 = sing.tile([P, NB], f32)
    c2 = sing.tile([P, NB], f32)
    x3 = sing.tile([P, NB], f32)
    t3 = sing.tile([P, NB], f32)
    ca = sing.tile([P, 1], f32)
    cb = sing.tile([P, 1], f32)

    # c1 = (dr1 - dn1 > 0) ? (dr1 - dn1 + MARGIN2) : 0
    nc.vector.tensor_tensor(out=x1[:], in0=dA[:, :, 0], in1=dA[:, :, 1], op=Alu.subtract)
    nc.vector.tensor_scalar(
        out=m1[:], in0=x1[:], scalar1=0.0, scalar2=None, op0=Alu.is_gt
    )
    nc.vector.scalar_tensor_tensor(
        out=c1[:], in0=x1[:], scalar=MARGIN2, in1=m1[:],
        op0=Alu.add, op1=Alu.mult, accum_out=ca[:],
    )
    # c2 = relu(dn2 - dr2)
    nc.vector.tensor_tensor(out=x2[:], in0=dV[:, :, 0], in1=dA[:, :, 2], op=Alu.subtract)
    nc.vector.tensor_scalar(
        out=c2[:], in0=x2[:], scalar1=0.0, scalar2=None,
        op0=Alu.max, op1=Alu.add, accum_out=cb[:],
    )
    # t = relu(tp - tn)
    nc.vector.tensor_tensor(out=x3[:], in0=dV[:, :, 1], in1=dV[:, :, 2], op=Alu.subtract)
    nc.vector.tensor_scalar(
        out=t3[:], in0=x3[:], scalar1=0.0, scalar2=None,
        op0=Alu.max, op1=Alu.add, accum_out=part[:, 3:4],
    )
    nc.vector.tensor_tensor(out=part[:, 2:3], in0=ca[:], in1=cb[:], op=Alu.add)
    nc.sync.dma_start(out=pd[:], in_=part[:])


_COMPILED = None


def _build():
    global _COMPILED
    if _COMPILED is not None:
        return _COMPILED
    nc = bacc.Bacc(
        "TRN2",
        target_bir_lowering=False,
        debug=False,
        enable_asserts=False,
        num_devices=NCORES,
    )
    ins = {
        "xout": nc.dram_tensor("xout", [RS, C], f32, kind="ExternalInput").ap(),
        "anc": nc.dram_tensor("anc", [BS, D], f32, kind="ExternalInput").ap(),
        "pos": nc.dram_tensor("pos", [BS, D], f32, kind="ExternalInput").ap(),
        "neg": nc.dram_tensor("neg", [BS, D], f32, kind="ExternalInput").ap(),
        "exem": nc.dram_tensor("exem", [C, D], f32, kind="ExternalInput").ap(),
        "lab_a": nc.dram_tensor("lab_a", [P, NB], i32, kind="ExternalInput").ap(),
        "lab_n": nc.dram_tensor("lab_n", [P, NB], i32, kind="ExternalInput").ap(),
        "lab_f": nc.dram_tensor("lab_f", [P, NR], f32, kind="ExternalInput").ap(),
    }
    outs = {
        "partials": nc.dram_tensor("partials", [P, 4], f32, kind="ExternalOutput").ap()
    }
    with tile.TileContext(nc) as tc:
        _emit(tc, outs, ins)
    nc.compile()
    _COMPILED = nc
    return nc


def _in_maps(anchor, positive, negative, outputs, labels_anchor, labels_neg, exemplars):
    anchor = np.asarray(anchor, np.float32)
    positive = np.asarray(positive, np.float32)
    negative = np.asarray(negative, np.float32)
    outputs = np.asarray(outputs, np.float32)
    exemplars = np.ascontiguousarray(np.asarray(exemplars, np.float32))
    la_all = np.asarray(labels_anchor).astype(np.int64)
    ln_all = np.asarray(labels_neg).astype(np.int64)

    maps = []
    for k in range(NCORES):
        sl = slice(k * BS, (k + 1) * BS)
        la, ln = la_all[sl], ln_all[sl]
        xo = np.ascontiguousarray(
            np.concatenate(
                [
                    outputs[k * BS : (k + 1) * BS],
                    outputs[B + k * BS : B + (k + 1) * BS],
                    outputs[2 * B + k * BS : 2 * B + (k + 1) * BS],
                ],
                axis=0,
            )
        )
        labels_cat = np.concatenate([la, la, ln])
        maps.append(
            {
                "xout": xo,
                "anc": np.ascontiguousarray(anchor[sl]),
                "pos": np.ascontiguousarray(positive[sl]),
                "neg": np.ascontiguousarray(negative[sl]),
                "exem": exemplars,
                "lab_a": np.ascontiguousarray(la.reshape(NB, P).T.astype(np.int32)),
                "lab_n": np.ascontiguousarray(ln.reshape(NB, P).T.astype(np.int32)),
                "lab_f": np.ascontiguousarray(
                    labels_cat.reshape(NR, P).T.astype(np.float32)
                ),
            }
        )
    return maps


def _combine(results):
    S = np.zeros(4, dtype=np.float64)
    for r in results:
        S += r["partials"].astype(np.float64).sum(axis=0)
    loss_softmax = (S[0] - S[1]) / (3 * B)
    loss_center = S[2]
    loss_triplet = S[3]
    loss_total = loss_softmax + 0.01 * loss_center + LAMBDA * loss_triplet
    return (
        np.float32(loss_total),
        np.float32(loss_triplet),
        np.float32(loss_softmax),
        np.float32(loss_center),
    )


def kernel(anchor, positive, negative, outputs, labels_anchor, labels_neg, exemplars):
    global LAST_RESULTS
    nc = _build()
    maps = _in_maps(
        anchor, positive, negative, outputs, labels_anchor, labels_neg, exemplars
    )
    res = run_bass_kernel_spmd(nc, maps, core_ids=list(range(NCORES)))
    LAST_RESULTS = res
    return _combine(res.results)


# revision 22
# speedup vs baseline: 1.0087x; 1.0087x over previous
"""Trainium2 Bass kernel for nn_ExemplarSoftmaxLoss (data-parallel over 8 cores).

Strategy:
  - Shard batch dim B (and the 3 B-row blocks of `outputs`) across 8 cores.
  - Per core, on device:
      * softmax part: per-row sum(exp(x)) via ScalarE Exp with row-accumulate
        (no max subtraction needed: |x| <= ~6 so exp is safely in fp32 range);
        label logits extracted on VectorE with a fused
        (iota == label) * x row-sum (scalar_tensor_tensor with accum_out).
      * distance part: exemplar rows gathered per 128-row block with
        [128,1]-offset indirect DMAs (the only HW-verified gather pattern),
        diffs (x + eps - y) on VectorE, squared-row-sums on ScalarE
        (Square w/ accumulate), sqrt + hinge logic on tiny [128,16] tiles.
      * outputs per-partition partial sums [128, 4].
  - Host: float64 reduction of the 8x[128,4] partials -> 4 scalar losses.
"""

import os
import sys

import numpy as np

for _p in ("/opt/trn_rl_repo",):
    if _p not in sys.path and os.path.isdir(_p):
        sys.path.insert(0, _p)

import concourse.bass as bass
import concourse.tile as tile
from concourse import bacc, mybir
from concourse._compat import with_exitstack
from concourse.bass_utils import run_bass_kernel_spmd

# Problem constants (hardcoded per the harness contract).
B, D, C = 16384, 512, 1000
NCORES = 8
BS = B // NCORES  # 2048 batch rows per core
RS = 3 * BS  # 6144 softmax rows per core
P = 128
NB = BS // P  # 16 row-blocks in the distance phase
NR = RS // P  # 48 row-blocks in the softmax phase
NG = 4  # groups of 4 row-blocks in the distance phase
EPS = 1e-6
MARGIN2 = 0.2
LAMBDA = 1.0

f32 = mybir.dt.float32
i32 = mybir.dt.int32
Alu = mybir.AluOpType
Act = mybir.ActivationFunctionType
AX = mybir.AxisListType

LAST_RESULTS = None  # BassKernelResults of the most recent run (for test.py)


@with_exitstack
def _emit(ctx, tc, outs, ins):
    nc = tc.nc
    xo = ins["xout"]  # [RS, C]   f32 outputs shard (3 blocks concatenated)
    aa = ins["anc"]  # [BS, D]   f32
    pp = ins["pos"]  # [BS, D]   f32
    ng = ins["neg"]  # [BS, D]   f32
    ex = ins["exem"]  # [C, D]    f32 exemplar table
    la = ins["lab_a"]  # [P, NB]  i32  labels_anchor, row blk*128+p at [p, blk]
    ln = ins["lab_n"]  # [P, NB]  i32  labels_neg
    lf = ins["lab_f"]  # [P, NR]  f32  concat labels as f32, row rb*128+p at [p, rb]
    pd = outs["partials"]  # [P, 4]  f32

    sing = ctx.enter_context(tc.tile_pool(name="sing", bufs=1))
    xpool = ctx.enter_context(tc.tile_pool(name="xp", bufs=5))
    ejp = ctx.enter_context(tc.tile_pool(name="ejp", bufs=2, space="PSUM"))
    ljp = ctx.enter_context(tc.tile_pool(name="ljp", bufs=2))
    apnp = ctx.enter_context(tc.tile_pool(name="apnp", bufs=2))
    expool = ctx.enter_context(tc.tile_pool(name="expool", bufs=2))
    dfp = ctx.enter_context(tc.tile_pool(name="dfp", bufs=3))
    sqp = ctx.enter_context(tc.tile_pool(name="sqp", bufs=3, space="PSUM"))
    sqvp = ctx.enter_context(tc.tile_pool(name="sqvp", bufs=3))

    sums = sing.tile([P, NR], f32)  # per-row sum(exp(x))
    lbl = sing.tile([P, NR], f32)  # extracted label logits
    d2a = sing.tile([P, NB * 3], f32)  # sq dists: dr1,dn1,dr2
    d2v = sing.tile([P, NB * 3], f32)  # sq dists: dn2,tp,tn
    la_t = sing.tile([P, NB], i32)
    ln_t = sing.tile([P, NB], i32)
    lf_t = sing.tile([P, NR], f32)
    iota_t = sing.tile([P, C], f32)

    # small loads via SWDGE so the Sync HWDGE queue leads with the x-tile stream
    nc.gpsimd.dma_start(out=la_t[:], in_=la[:])
    nc.gpsimd.dma_start(out=ln_t[:], in_=ln[:])
    nc.gpsimd.dma_start(out=lf_t[:], in_=lf[:])
    nc.gpsimd.iota(
        iota_t[:],
        pattern=[[1, C]],
        base=0,
        channel_multiplier=0,
        allow_small_or_imprecise_dtypes=True,
    )

    def emit_gathers(g, exa, exn, b2s=range(4)):
        for b2 in b2s:
            blk = 4 * g + b2
            nc.gpsimd.indirect_dma_start(
                out=exa[:, b2, :],
                out_offset=None,
                in_=ex[:],
                in_offset=bass.IndirectOffsetOnAxis(ap=la_t[:, blk : blk + 1], axis=0),
            )
            nc.gpsimd.indirect_dma_start(
                out=exn[:, b2, :],
                out_offset=None,
                in_=ex[:],
                in_offset=bass.IndirectOffsetOnAxis(ap=ln_t[:, blk : blk + 1], axis=0),
            )

    def emit_apn_loads(g):
        at = apnp.tile([P, 4, D], f32, tag="at", name=f"at{g}")
        pt = apnp.tile([P, 4, D], f32, tag="pt", name=f"pt{g}")
        nt = apnp.tile([P, 4, D], f32, tag="nt", name=f"nt{g}")
        r0, r1 = g * 4 * P, (g + 1) * 4 * P
        nc.sync.dma_start(
            out=at[:], in_=aa[r0:r1, :].rearrange("(t p) d -> p t d", p=P)
        )
        nc.sync.dma_start(
            out=pt[:], in_=pp[r0:r1, :].rearrange("(t p) d -> p t d", p=P)
        )
        nc.sync.dma_start(
            out=nt[:], in_=ng[r0:r1, :].rearrange("(t p) d -> p t d", p=P)
        )
        return at, pt, nt

    # software-pipeline the exemplar gathers + anchor/pos/neg loads one group
    # ahead so SWDGE descriptor generation and DMA overlap compute
    ex_tiles = {
        0: (
            expool.tile([P, 4, D], f32, tag="exa", name="exa0"),
            expool.tile([P, 4, D], f32, tag="exn", name="exn0"),
        )
    }
    emit_gathers(0, *ex_tiles[0])
    apn_tiles = {}

    for g in range(NG):
        exa, exn = ex_tiles.pop(g)
        # interleave 1 x-tile : 1 distance pair for smooth per-engine FIFOs
        for pi in range(6):
            xi = 6 * g + pi
            xt = xpool.tile([P, 2, C], f32, tag="xt")
            nc.sync.dma_start(
                out=xt[:],
                in_=xo[xi * 2 * P : (xi + 1) * 2 * P, :].rearrange(
                    "(t p) c -> p t c", p=P
                ),
            )
            for b in range(2):
                rb = xi * 2 + b
                ej = ejp.tile([P, C], f32, tag="ej")
                nc.scalar.activation(
                    out=ej[:],
                    in_=xt[:, b, :],
                    func=Act.Exp,
                    accum_out=sums[:, rb : rb + 1],
                )
                # lbl[p, rb] = sum((iota == label) * x) = x[p, label]
                lj = ljp.tile([P, C], f32, tag="lj")
                nc.vector.scalar_tensor_tensor(
                    out=lj[:],
                    in0=iota_t[:],
                    scalar=lf_t[:, rb : rb + 1],
                    in1=xt[:, b, :],
                    op0=Alu.is_equal,
                    op1=Alu.mult,
                    accum_out=lbl[:, rb : rb + 1],
                )

            if g == 0 and pi == 0:
                # first group's apn loads queue behind the first x-tile
                apn_tiles[0] = emit_apn_loads(0)
            if pi == 2 and g + 1 < NG:
                ex_tiles[g + 1] = (
                    expool.tile([P, 4, D], f32, tag="exa", name=f"exa{g + 1}"),
                    expool.tile([P, 4, D], f32, tag="exn", name=f"exn{g + 1}"),
                )
            if pi == 3 and g + 1 < NG:
                # prefetch next group's apn mid-group (decongests the ramp)
                apn_tiles[g + 1] = emit_apn_loads(g + 1)
            if pi >= 2 and g + 1 < NG:
                # spread next group's gathers: 2 indirect DMAs per step
                emit_gathers(g + 1, *ex_tiles[g + 1], b2s=[pi - 2])

            if pi == 0:
                at, pt, nt = apn_tiles.pop(g)
                # squares: ~68 on ScalarE, ~28 on VectorE (measured balance);
                # dn2/tp partially alternate per group
                pairs = (
                    (at[:], exa[:], d2a, 0, True),  # d_ref1  -> ScalarE
                    (nt[:], exa[:], d2a, 1, True),  # d_neg1  -> ScalarE
                    (at[:], exn[:], d2a, 2, True),  # d_ref2  -> ScalarE
                    (nt[:], exn[:], d2v, 0, g != 3),  # d_neg2 -> 12/4
                    (at[:], pt[:], d2v, 1, g % 2 == 0),  # tp -> split 8/8
                    (at[:], nt[:], d2v, 2, False),  # tn    -> VectorE
                )
            xs, ys, d2t, ci, on_act = pairs[pi]

            df = dfp.tile([P, 4, D], f32, tag="df")
            # df = x - y.  (The reference's +EPS inside the norm shifts d^2
            # by ~2*EPS*|sum(diff)| ~ 1e-7 relative -- negligible.)
            nc.vector.tensor_tensor(out=df[:], in0=xs, in1=ys, op=Alu.subtract)
            for b2 in range(4):
                col = (4 * g + b2) * 3 + ci
                if on_act:
                    sq = sqp.tile([P, D], f32, tag="sqa")
                    nc.scalar.activation(
                        out=sq[:],
                        in_=df[:, b2, :],
                        func=Act.Square,
                        accum_out=d2t[:, col : col + 1],
                    )
                else:
                    sq = sqvp.tile([P, D], f32, tag="sqv")
                    # (df * 1.0) * df with sum-accumulate == row-sum of df^2
                    nc.vector.scalar_tensor_tensor(
                        out=sq[:],
                        in0=df[:, b2, :],
                        scalar=1.0,
                        in1=df[:, b2, :],
                        op0=Alu.mult,
                        op1=Alu.mult,
                        accum_out=d2t[:, col : col + 1],
                    )

    # ---- tail ----
    part = sing.tile([P, 4], f32)
    logs = sing.tile([P, NR], f32)
    nc.scalar.activation(out=logs[:], in_=sums[:], func=Act.Ln)
    nc.vector.reduce_sum(out=part[:, 0:1], in_=logs[:], axis=AX.X)
    nc.vector.reduce_sum(out=part[:, 1:2], in_=lbl[:], axis=AX.X)

    dda = sing.tile([P, NB * 3], f32)
    ddv = sing.tile([P, NB * 3], f32)
    nc.scalar.activation(out=dda[:], in_=d2a[:], func=Act.Sqrt)
    nc.scalar.activation(out=ddv[:], in_=d2v[:], func=Act.Sqrt)
    dA = dda[:].rearrange("p (b k) -> p b k", k=3)
    dV = ddv[:].rearrange("p (b k) -> p b k", k=3)

    x1 = sing.tile([P, NB], f32)
    m1 = sing.tile([P, NB], f32)
    c1 = sing.tile([P, NB], f32)
    x2 = sing.tile([P, NB], f32)
    c2 = sing.tile([P, NB], f32)
    x3 = sing.tile([P, NB], f32)
    t3 = sing.tile([P, NB], f32)
    ca = sing.tile([P, 1], f32)
    cb = sing.tile([P, 1], f32)

    # c1 = (dr1 - dn1 > 0) ? (dr1 - dn1 + MARGIN2) : 0
    nc.vector.tensor_tensor(out=x1[:], in0=dA[:, :, 0], in1=dA[:, :, 1], op=Alu.subtract)
    nc.vector.tensor_scalar(
        out=m1[:], in0=x1[:], scalar1=0.0, scalar2=None, op0=Alu.is_gt
    )
    nc.vector.scalar_tensor_tensor(
        out=c1[:], in0=x1[:], scalar=MARGIN2, in1=m1[:],
        op0=Alu.add, op1=Alu.mult, accum_out=ca[:],
    )
    # c2 = relu(dn2 - dr2)
    nc.vector.tensor_tensor(out=x2[:], in0=dV[:, :, 0], in1=dA[:, :, 2], op=Alu.subtract)
    nc.vector.tensor_scalar(
        out=c2[:], in0=x2[:], scalar1=0.0, scalar2=None,
        op0=Alu.max, op1=Alu.add, accum_out=cb[:],
    )
    # t = relu(tp - tn)
    nc.vector.tensor_tensor(out=x3[:], in0=dV[:, :, 1], in1=dV[:, :, 2], op=Alu.subtract)
    nc.vector.tensor_scalar(
        out=t3[:], in0=x3[:], scalar1=0.0, scalar2=None,
        op0=Alu.max, op1=Alu.add, accum_out=part[:, 3:4],
    )
    nc.vector.tensor_tensor(out=part[:, 2:3], in0=ca[:], in1=cb[:], op=Alu.add)
    nc.sync.dma_start(out=pd[:], in_=part[:])


_COMPILED = None


def _build():
    global _COMPILED
    if _COMPILED is not None:
        return _COMPILED
    nc = bacc.Bacc(
        "TRN2",
        target_bir_lowering=False,
        debug=False,
        enable_asserts=False,
        num_devices=NCORES,
    )
    ins = {
        "xout": nc.dram_tensor("xout", [RS, C], f32, kind="ExternalInput").ap(),
        "anc": nc.dram_tensor("anc", [BS, D], f32, kind="ExternalInput").ap(),
        "pos": nc.dram_tensor("pos", [BS, D], f32, kind="ExternalInput").ap(),
        "neg": nc.dram_tensor("neg", [BS, D], f32, kind="ExternalInput").ap(),
        "exem": nc.dram_tensor("exem", [C, D], f32, kind="ExternalInput").ap(),
        "lab_a": nc.dram_tensor("lab_a", [P, NB], i32, kind="ExternalInput").ap(),
        "lab_n": nc.dram_tensor("lab_n", [P, NB], i32, kind="ExternalInput").ap(),
        "lab_f": nc.dram_tensor("lab_f", [P, NR], f32, kind="ExternalInput").ap(),
    }
    outs = {
        "partials": nc.dram_tensor("partials", [P, 4], f32, kind="ExternalOutput").ap()
    }
    with tile.TileContext(nc) as tc:
        _emit(tc, outs, ins)
    nc.compile()
    _COMPILED = nc
    return nc


def _in_maps(anchor, positive, negative, outputs, labels_anchor, labels_neg, exemplars):
    anchor = np.asarray(anchor, np.float32)
    positive = np.asarray(positive, np.float32)
    negative = np.asarray(negative, np.float32)
    outputs = np.asarray(outputs, np.float32)
    exemplars = np.ascontiguousarray(np.asarray(exemplars, np.float32))
    la_all = np.asarray(labels_anchor).astype(np.int64)
    ln_all = np.asarray(labels_neg).astype(np.int64)

    maps = []
    for k in range(NCORES):
        sl = slice(k * BS, (k + 1) * BS)
        la, ln = la_all[sl], ln_all[sl]
        xo = np.ascontiguousarray(
            np.concatenate(
                [
                    outputs[k * BS : (k + 1) * BS],
                    outputs[B + k * BS : B + (k + 1) * BS],
                    outputs[2 * B + k * BS : 2 * B + (k + 1) * BS],
                ],
                axis=0,
            )
        )
        labels_cat = np.concatenate([la, la, ln])
        maps.append(
            {
                "xout": xo,
                "anc": np.ascontiguousarray(anchor[sl]),
                "pos": np.ascontiguousarray(positive[sl]),
                "neg": np.ascontiguousarray(negative[sl]),
                "exem": exemplars,
                "lab_a": np.ascontiguousarray(la.reshape(NB, P).T.astype(np.int32)),
                "lab_n": np.ascontiguousarray(ln.reshape(NB, P).T.astype(np.int32)),
                "lab_f": np.ascontiguousarray(
                    labels_cat.reshape(NR, P).T.astype(np.float32)
                ),
            }
        )
    return maps


def _combine(results):
    S = np.zeros(4, dtype=np.float64)
    for r in results:
        S += r["partials"].astype(np.float64).sum(axis=0)
    loss_softmax = (S[0] - S[1]) / (3 * B)
    loss_center = S[2]
    loss_triplet = S[3]
    loss_total = loss_softmax + 0.01 * loss_center + LAMBDA * loss_triplet
    return (
        np.float32(loss_total),
        np.float32(loss_triplet),
        np.float32(loss_softmax),
        np.float32(loss_center),
    )


def kernel(anchor, positive, negative, outputs, labels_anchor, labels_neg, exemplars):
    global LAST_RESULTS
    nc = _build()
    maps = _in_maps(
        anchor, positive, negative, outputs, labels_anchor, labels_neg, exemplars
    )
    res = run_bass_kernel_spmd(nc, maps, core_ids=list(range(NCORES)))
    LAST_RESULTS = res
    return _combine(res.results)


# revision 23
# speedup vs baseline: 1.0546x; 1.0455x over previous
"""Trainium2 Bass kernel for nn_ExemplarSoftmaxLoss (data-parallel over 8 cores).

Strategy:
  - Shard batch dim B (and the 3 B-row blocks of `outputs`) across 8 cores.
  - Per core, on device:
      * softmax part: per-row sum(exp(x)) via ScalarE Exp with row-accumulate
        (no max subtraction needed: |x| <= ~6 so exp is safely in fp32 range);
        label logits extracted on VectorE with a fused
        (iota == label) * x row-sum (scalar_tensor_tensor with accum_out).
      * distance part: exemplar rows gathered per 128-row block with
        [128,1]-offset indirect DMAs (the only HW-verified gather pattern),
        diffs (x + eps - y) on VectorE, squared-row-sums on ScalarE
        (Square w/ accumulate), sqrt + hinge logic on tiny [128,16] tiles.
      * outputs per-partition partial sums [128, 4].
  - Host: float64 reduction of the 8x[128,4] partials -> 4 scalar losses.
"""

import os
import sys

import numpy as np

for _p in ("/opt/trn_rl_repo",):
    if _p not in sys.path and os.path.isdir(_p):
        sys.path.insert(0, _p)

import concourse.bass as bass
import concourse.tile as tile
from concourse import bacc, mybir
from concourse._compat import with_exitstack
from concourse.bass_utils import run_bass_kernel_spmd

# If BASS_TRACE is set in the environment, run_bass_kernel_spmd imports
# antenv.axon_hooks, which this image lacks -- stub it so we degrade to
# an untraced run instead of crashing.
try:
    import antenv.axon_hooks  # noqa: F401
except ImportError:
    import types as _types

    _m = _types.ModuleType("antenv.axon_hooks")
    _m.get_axon_ntff_profile_hook = lambda: None
    _m.set_axon_ntff_profile_hook = lambda h: None
    sys.modules["antenv.axon_hooks"] = _m

# Problem constants (hardcoded per the harness contract).
B, D, C = 16384, 512, 1000
NCORES = 8
BS = B // NCORES  # 2048 batch rows per core
RS = 3 * BS  # 6144 softmax rows per core
P = 128
NB = BS // P  # 16 row-blocks in the distance phase
NR = RS // P  # 48 row-blocks in the softmax phase
NG = 4  # groups of 4 row-blocks in the distance phase
EPS = 1e-6
MARGIN2 = 0.2
LAMBDA = 1.0

f32 = mybir.dt.float32
i32 = mybir.dt.int32
Alu = mybir.AluOpType
Act = mybir.ActivationFunctionType
AX = mybir.AxisListType

LAST_RESULTS = None  # BassKernelResults of the most recent run (for test.py)


@with_exitstack
def _emit(ctx, tc, outs, ins):
    nc = tc.nc
    xo = ins["xout"]  # [RS, C]   f32 outputs shard (3 blocks concatenated)
    aa = ins["anc"]  # [BS, D]   f32
    pp = ins["pos"]  # [BS, D]   f32
    ng = ins["neg"]  # [BS, D]   f32
    ex = ins["exem"]  # [C, D]    f32 exemplar table
    la = ins["lab_a"]  # [P, NB]  i32  labels_anchor, row blk*128+p at [p, blk]
    ln = ins["lab_n"]  # [P, NB]  i32  labels_neg
    lf = ins["lab_f"]  # [P, NR]  f32  concat labels as f32, row rb*128+p at [p, rb]
    pd = outs["partials"]  # [P, 4]  f32

    sing = ctx.enter_context(tc.tile_pool(name="sing", bufs=1))
    xpool = ctx.enter_context(tc.tile_pool(name="xp", bufs=5))
    ejp = ctx.enter_context(tc.tile_pool(name="ejp", bufs=2, space="PSUM"))
    ljp = ctx.enter_context(tc.tile_pool(name="ljp", bufs=2))
    apnp = ctx.enter_context(tc.tile_pool(name="apnp", bufs=2))
    expool = ctx.enter_context(tc.tile_pool(name="expool", bufs=2))
    dfp = ctx.enter_context(tc.tile_pool(name="dfp", bufs=3))
    sqp = ctx.enter_context(tc.tile_pool(name="sqp", bufs=3, space="PSUM"))
    sqvp = ctx.enter_context(tc.tile_pool(name="sqvp", bufs=3))

    sums = sing.tile([P, NR], f32)  # per-row sum(exp(x))
    lbl = sing.tile([P, NR], f32)  # extracted label logits
    d2a = sing.tile([P, NB * 3], f32)  # sq dists: dr1,dn1,dr2
    d2v = sing.tile([P, NB * 3], f32)  # sq dists: dn2,tp,tn
    la_t = sing.tile([P, NB], i32)
    ln_t = sing.tile([P, NB], i32)
    lf_t = sing.tile([P, NR], f32)
    iota_t = sing.tile([P, C], f32)

    # small loads via SWDGE so the Sync HWDGE queue leads with the x-tile stream
    nc.gpsimd.dma_start(out=la_t[:], in_=la[:])
    nc.gpsimd.dma_start(out=ln_t[:], in_=ln[:])
    nc.gpsimd.dma_start(out=lf_t[:], in_=lf[:])
    nc.gpsimd.iota(
        iota_t[:],
        pattern=[[1, C]],
        base=0,
        channel_multiplier=0,
        allow_small_or_imprecise_dtypes=True,
    )

    def emit_gathers(g, exa, exn, b2s=range(4)):
        for b2 in b2s:
            blk = 4 * g + b2
            nc.gpsimd.indirect_dma_start(
                out=exa[:, b2, :],
                out_offset=None,
                in_=ex[:],
                in_offset=bass.IndirectOffsetOnAxis(ap=la_t[:, blk : blk + 1], axis=0),
            )
            nc.gpsimd.indirect_dma_start(
                out=exn[:, b2, :],
                out_offset=None,
                in_=ex[:],
                in_offset=bass.IndirectOffsetOnAxis(ap=ln_t[:, blk : blk + 1], axis=0),
            )

    def emit_apn_loads(g):
        at = apnp.tile([P, 4, D], f32, tag="at", name=f"at{g}")
        pt = apnp.tile([P, 4, D], f32, tag="pt", name=f"pt{g}")
        nt = apnp.tile([P, 4, D], f32, tag="nt", name=f"nt{g}")
        r0, r1 = g * 4 * P, (g + 1) * 4 * P
        nc.sync.dma_start(
            out=at[:], in_=aa[r0:r1, :].rearrange("(t p) d -> p t d", p=P)
        )
        nc.sync.dma_start(
            out=pt[:], in_=pp[r0:r1, :].rearrange("(t p) d -> p t d", p=P)
        )
        nc.sync.dma_start(
            out=nt[:], in_=ng[r0:r1, :].rearrange("(t p) d -> p t d", p=P)
        )
        return at, pt, nt

    # software-pipeline the exemplar gathers + anchor/pos/neg loads one group
    # ahead so SWDGE descriptor generation and DMA overlap compute
    ex_tiles = {
        0: (
            expool.tile([P, 4, D], f32, tag="exa", name="exa0"),
            expool.tile([P, 4, D], f32, tag="exn", name="exn0"),
        )
    }
    emit_gathers(0, *ex_tiles[0])
    apn_tiles = {}

    for g in range(NG):
        exa, exn = ex_tiles.pop(g)
        # interleave 1 x-tile : 1 distance pair for smooth per-engine FIFOs
        for pi in range(6):
            xi = 6 * g + pi
            xt = xpool.tile([P, 2, C], f32, tag="xt")
            nc.sync.dma_start(
                out=xt[:],
                in_=xo[xi * 2 * P : (xi + 1) * 2 * P, :].rearrange(
                    "(t p) c -> p t c", p=P
                ),
            )
            for b in range(2):
                rb = xi * 2 + b
                ej = ejp.tile([P, C], f32, tag="ej")
                nc.scalar.activation(
                    out=ej[:],
                    in_=xt[:, b, :],
                    func=Act.Exp,
                    accum_out=sums[:, rb : rb + 1],
                )
                # lbl[p, rb] = sum((iota == label) * x) = x[p, label]
                lj = ljp.tile([P, C], f32, tag="lj")
                nc.vector.scalar_tensor_tensor(
                    out=lj[:],
                    in0=iota_t[:],
                    scalar=lf_t[:, rb : rb + 1],
                    in1=xt[:, b, :],
                    op0=Alu.is_equal,
                    op1=Alu.mult,
                    accum_out=lbl[:, rb : rb + 1],
                )

            if g == 0 and pi == 0:
                # first group's apn loads queue behind the first x-tile
                apn_tiles[0] = emit_apn_loads(0)
            if pi == 2 and g + 1 < NG:
                ex_tiles[g + 1] = (
                    expool.tile([P, 4, D], f32, tag="exa", name=f"exa{g + 1}"),
                    expool.tile([P, 4, D], f32, tag="exn", name=f"exn{g + 1}"),
                )
            if pi == 3 and g + 1 < NG:
                # prefetch next group's apn mid-group (decongests the ramp)
                apn_tiles[g + 1] = emit_apn_loads(g + 1)
            if pi >= 2 and g + 1 < NG:
                # spread next group's gathers: 2 indirect DMAs per step
                emit_gathers(g + 1, *ex_tiles[g + 1], b2s=[pi - 2])

            if pi == 0:
                at, pt, nt = apn_tiles.pop(g)
                # squares: ~68 on ScalarE, ~28 on VectorE (measured balance);
                # dn2/tp partially alternate per group
                pairs = (
                    (at[:], exa[:], d2a, 0, True),  # d_ref1  -> ScalarE
                    (nt[:], exa[:], d2a, 1, True),  # d_neg1  -> ScalarE
                    (at[:], exn[:], d2a, 2, True),  # d_ref2  -> ScalarE
                    (nt[:], exn[:], d2v, 0, g != 3),  # d_neg2 -> 12/4
                    (at[:], pt[:], d2v, 1, g % 2 == 0),  # tp -> split 8/8
                    (at[:], nt[:], d2v, 2, False),  # tn    -> VectorE
                )
            xs, ys, d2t, ci, on_act = pairs[pi]

            df = dfp.tile([P, 4, D], f32, tag="df")
            # df = x - y.  (The reference's +EPS inside the norm shifts d^2
            # by ~2*EPS*|sum(diff)| ~ 1e-7 relative -- negligible.)
            nc.vector.tensor_tensor(out=df[:], in0=xs, in1=ys, op=Alu.subtract)
            for b2 in range(4):
                col = (4 * g + b2) * 3 + ci
                if on_act:
                    sq = sqp.tile([P, D], f32, tag="sqa")
                    nc.scalar.activation(
                        out=sq[:],
                        in_=df[:, b2, :],
                        func=Act.Square,
                        accum_out=d2t[:, col : col + 1],
                    )
                else:
                    sq = sqvp.tile([P, D], f32, tag="sqv")
                    # (df * 1.0) * df with sum-accumulate == row-sum of df^2
                    nc.vector.scalar_tensor_tensor(
                        out=sq[:],
                        in0=df[:, b2, :],
                        scalar=1.0,
                        in1=df[:, b2, :],
                        op0=Alu.mult,
                        op1=Alu.mult,
                        accum_out=d2t[:, col : col + 1],
                    )

    # ---- tail ----
    part = sing.tile([P, 4], f32)
    logs = sing.tile([P, NR], f32)
    nc.scalar.activation(out=logs[:], in_=sums[:], func=Act.Ln)
    nc.vector.reduce_sum(out=part[:, 0:1], in_=logs[:], axis=AX.X)
    nc.vector.reduce_sum(out=part[:, 1:2], in_=lbl[:], axis=AX.X)

    dda = sing.tile([P, NB * 3], f32)
    ddv = sing.tile([P, NB * 3], f32)
    nc.scalar.activation(out=dda[:], in_=d2a[:], func=Act.Sqrt)
    nc.scalar.activation(out=ddv[:], in_=d2v[:], func=Act.Sqrt)
    dA = dda[:].rearrange("p (b k) -> p b k", k=3)
    dV = ddv[:].rearrange("p (b k) -> p b k", k=3)

    x1 = sing.tile([P, NB], f32)
    m1 = sing.tile([P, NB], f32)
    c1 = sing.tile([P, NB], f32)
    x2 = sing.tile([P, NB], f32)
    c2 = sing.tile([P, NB], f32)
    x3 = sing.tile([P, NB], f32)
    t3 = sing.tile([P, NB], f32)
    ca = sing.tile([P, 1], f32)
    cb = sing.tile([P, 1], f32)

    # c1 = (dr1 - dn1 > 0) ? (dr1 - dn1 + MARGIN2) : 0
    nc.vector.tensor_tensor(out=x1[:], in0=dA[:, :, 0], in1=dA[:, :, 1], op=Alu.subtract)
    nc.vector.tensor_scalar(
        out=m1[:], in0=x1[:], scalar1=0.0, scalar2=None, op0=Alu.is_gt
    )
    nc.vector.scalar_tensor_tensor(
        out=c1[:], in0=x1[:], scalar=MARGIN2, in1=m1[:],
        op0=Alu.add, op1=Alu.mult, accum_out=ca[:],
    )
    # c2 = relu(dn2 - dr2)
    nc.vector.tensor_tensor(out=x2[:], in0=dV[:, :, 0], in1=dA[:, :, 2], op=Alu.subtract)
    nc.vector.tensor_scalar(
        out=c2[:], in0=x2[:], scalar1=0.0, scalar2=None,
        op0=Alu.max, op1=Alu.add, accum_out=cb[:],
    )
    # t = relu(tp - tn)
    nc.vector.tensor_tensor(out=x3[:], in0=dV[:, :, 1], in1=dV[:, :, 2], op=Alu.subtract)
    nc.vector.tensor_scalar(
        out=t3[:], in0=x3[:], scalar1=0.0, scalar2=None,
        op0=Alu.max, op1=Alu.add, accum_out=part[:, 3:4],
    )
    nc.vector.tensor_tensor(out=part[:, 2:3], in0=ca[:], in1=cb[:], op=Alu.add)
    nc.sync.dma_start(out=pd[:], in_=part[:])


_COMPILED = None


def _build():
    global _COMPILED
    if _COMPILED is not None:
        return _COMPILED
    nc = bacc.Bacc(
        "TRN2",
        target_bir_lowering=False,
        debug=False,
        enable_asserts=False,
        num_devices=NCORES,
    )
    ins = {
        "xout": nc.dram_tensor("xout", [RS, C], f32, kind="ExternalInput").ap(),
        "anc": nc.dram_tensor("anc", [BS, D], f32, kind="ExternalInput").ap(),
        "pos": nc.dram_tensor("pos", [BS, D], f32, kind="ExternalInput").ap(),
        "neg": nc.dram_tensor("neg", [BS, D], f32, kind="ExternalInput").ap(),
        "exem": nc.dram_tensor("exem", [C, D], f32, kind="ExternalInput").ap(),
        "lab_a": nc.dram_tensor("lab_a", [P, NB], i32, kind="ExternalInput").ap(),
        "lab_n": nc.dram_tensor("lab_n", [P, NB], i32, kind="ExternalInput").ap(),
        "lab_f": nc.dram_tensor("lab_f", [P, NR], f32, kind="ExternalInput").ap(),
    }
    outs = {
        "partials": nc.dram_tensor("partials", [P, 4], f32, kind="ExternalOutput").ap()
    }
    with tile.TileContext(nc) as tc:
        _emit(tc, outs, ins)
    nc.compile()
    _COMPILED = nc
    return nc


def _in_maps(anchor, positive, negative, outputs, labels_anchor, labels_neg, exemplars):
    anchor = np.asarray(anchor, np.float32)
    positive = np.asarray(positive, np.float32)
    negative = np.asarray(negative, np.float32)
    outputs = np.asarray(outputs, np.float32)
    exemplars = np.ascontiguousarray(np.asarray(exemplars, np.float32))
    la_all = np.asarray(labels_anchor).astype(np.int64)
    ln_all = np.asarray(labels_neg).astype(np.int64)

    maps = []
    for k in range(NCORES):
        sl = slice(k * BS, (k + 1) * BS)
        la, ln = la_all[sl], ln_all[sl]
        xo = np.ascontiguousarray(
            np.concatenate(
                [
                    outputs[k * BS : (k + 1) * BS],
                    outputs[B + k * BS : B + (k + 1) * BS],
                    outputs[2 * B + k * BS : 2 * B + (k + 1) * BS],
                ],
                axis=0,
            )
        )
        labels_cat = np.concatenate([la, la, ln])
        maps.append(
            {
                "xout": xo,
                "anc": np.ascontiguousarray(anchor[sl]),
                "pos": np.ascontiguousarray(positive[sl]),
                "neg": np.ascontiguousarray(negative[sl]),
                "exem": exemplars,
                "lab_a": np.ascontiguousarray(la.reshape(NB, P).T.astype(np.int32)),
                "lab_n": np.ascontiguousarray(ln.reshape(NB, P).T.astype(np.int32)),
                "lab_f": np.ascontiguousarray(
                    labels_cat.reshape(NR, P).T.astype(np.float32)
                ),
            }
        )
    return maps


def _combine(results):
    S = np.zeros(4, dtype=np.float64)
    for r in results:
        S += r["partials"].astype(np.float64).sum(axis=0)
    loss_softmax = (S[0] - S[1]) / (3 * B)
    loss_center = S[2]
    loss_triplet = S[3]
    loss_total = loss_softmax + 0.01 * loss_center + LAMBDA * loss_triplet
    return (
        np.float32(loss_total),
        np.float32(loss_triplet),
        np.float32(loss_softmax),
        np.float32(loss_center),
    )


def kernel(anchor, positive, negative, outputs, labels_anchor, labels_neg, exemplars):
    global LAST_RESULTS
    nc = _build()
    maps = _in_maps(
        anchor, positive, negative, outputs, labels_anchor, labels_neg, exemplars
    )
    res = run_bass_kernel_spmd(nc, maps, core_ids=list(range(NCORES)))
    LAST_RESULTS = res
    return _combine(res.results)
